# revision 5
# baseline (speedup 1.0000x reference)
"""Multi-head causal self-attention on 8 Trainium2 NeuronCores — v2.

Sharding: core c -> batch b = c // 2, heads 4*(c % 2) .. +4 (data parallel on
B, tensor parallel on heads).  Host sums the two partials per batch + b_out
(+ b_v @ W_out, since softmax weights sum to 1).

v2 redesign vs v1 (240us -> target ~65us):
- exp is split across ScalarE (exact, table exp) and VectorE (Schraudolph
  affine-int16 bit trick writing bf16 bits directly), assigned greedily by a
  build-time load balancer.  ACT was the 92us bottleneck in v1.
- softmax denominator: ones-column in V (M=65 AV matmuls, as v1), but the
  normalization chain is restructured: ACT/DVE evacuate unnormalized attn+den
  to SBUF f32, DMA hops the two heads' den rows into partitions 0/64 of one
  tile, ONE DVE reciprocal_approx_fast covers both, a K=1 PE outer-product
  broadcasts each reciprocal row into a freed PSUM slot, and one DVE multiply
  per head writes normalized bf16 attn.  (v1 did full-tile copy + full-tile
  reciprocal + gpsimd broadcast: ~28us DVE + 17us gpsimd.)
- out-projection: heads pair-packed into K=128 matmuls (v1: 64 K=64 MMs).
- causal mask: bf16 in-place triu multiply (2x DVE mode, 127ns vs 194ns f32).
- evacuations (qkv bias-adds, v, y, uattn) are engine-assignable jobs placed
  on whichever of ACT/DVE has less queued work.
- emission order keeps PE dense (HAM warm): background jobs (v proj, qk m=1,
  out-proj qb0) are woven between attention kt steps.
"""

import os
import sys
from contextlib import ExitStack

import numpy as np

for _p in ("/opt/trn_rl_repo", "/opt/pypackages"):
    if os.path.isdir(_p) and _p not in sys.path:
        sys.path.append(_p)

import concourse.bass as bass
from concourse import bacc
import concourse.mybir as mybir
import concourse.tile as tile
from concourse.bass_utils import run_bass_kernel_spmd
from concourse.masks import make_upper_triangular

B, T, D = 4, 2048, 512
H, HD = 8, 64
HPC = 4  # heads per core
P = 128
KT = D // P
AB = 1024  # attention query block
NQB = T // AB
NKT = T // P
VW = HD + 1  # v columns per head incl. ones column

F32 = mybir.dt.float32
BF16 = mybir.dt.bfloat16
I16 = mybir.dt.int16
EXP = mybir.ActivationFunctionType.Exp
IDENT = mybir.ActivationFunctionType.Identity

# Schraudolph exp into bf16 bits: i16 = rne(s_raw * SCH_A + SCH_B);
# bitcast(i16) ~ exp(s_raw / 8).  c = 0.043 minimizes max rel err (~3.3%).
SCH_A = 128.0 * np.log2(np.e) / 8.0
SCH_B = 128.0 * (127.0 - 0.043)

try:
    import ml_dtypes
    _NP_BF16 = np.dtype(ml_dtypes.bfloat16)
except ImportError:
    _NP_BF16 = np.float32


class Assigner:
    """Greedy ACT/DVE load balancer (build-time, microseconds)."""

    def __init__(self):
        self.t_act = 0.0
        self.t_dve = 0.0

    def pick(self, cost_act, cost_dve):
        if self.t_act + cost_act <= self.t_dve + cost_dve:
            self.t_act += cost_act
            return "act"
        self.t_dve += cost_dve
        return "dve"

    def exp_cost(self, cols):
        return (cols + 352) / 1200.0, (cols + 120) / 960.0

    def copy_cost(self, cols):
        return (cols + 172) / 1200.0, (cols + 120) / 960.0


def build_bass():
    nc = bacc.Bacc()
    # weights arrive pre-rearranged from the host ([p, kt, m] layouts) so
    # every weight DMA is a contiguous 2KB-per-partition transfer instead of
    # a strided 512B-line gather.
    xT = nc.declare_dram_parameter("xT", [D, T], BF16, isOutput=False)
    wqa = nc.declare_dram_parameter("wqa", [P, KT * 2 * P], BF16, isOutput=False)
    wka = nc.declare_dram_parameter("wka", [P, KT * 2 * P], BF16, isOutput=False)
    wqkb = nc.declare_dram_parameter("wqkb", [P, 4], F32, isOutput=False)
    wva = nc.declare_dram_parameter("wva", [P, KT * HPC * VW], BF16, isOutput=False)
    wo = nc.declare_dram_parameter("wo", [P, 2 * D], BF16, isOutput=False)
    y = nc.declare_dram_parameter("y", [T, D], F32, isOutput=True)

    asn = Assigner()

    with tile.TileContext(nc) as tc, ExitStack() as ctx:
        consts = ctx.enter_context(tc.tile_pool(name="consts", bufs=1))
        qkv = ctx.enter_context(tc.tile_pool(name="qkv", bufs=1))
        work = ctx.enter_context(tc.tile_pool(name="work", bufs=1))
        scps = ctx.enter_context(tc.tile_pool(name="scps", bufs=2, space="PSUM"))
        avps = ctx.enter_context(tc.tile_pool(name="avps", bufs=1, space="PSUM"))

        # ---- constants / inputs
        _salt = consts.tile([1, 8], F32, name="salt")
        nc.vector.memset(_salt, float(os.environ.get("MHSA_SALT", "7")))
        # warm the ACT exp table early (table load ~2.7us overlaps QKV ramp)
        nc.scalar.activation(out=_salt, in_=_salt, func=EXP, scale=0.01)

        # first-needed-first, all contiguous: k/q weights + biases (the ramp
        # matmuls' stationaries), then x, then v weights, then wo.
        wk_sb = consts.tile([P, KT, 2 * P], BF16)
        nc.sync.dma_start(
            out=wk_sb, in_=wka.rearrange("p (kt m) -> p kt m", m=2 * P)
        )
        wq_sb = consts.tile([P, KT, 2 * P], BF16)
        nc.sync.dma_start(
            out=wq_sb, in_=wqa.rearrange("p (kt m) -> p kt m", m=2 * P)
        )
        wqkb_sb = consts.tile([P, 4], F32)
        nc.sync.dma_start(out=wqkb_sb, in_=wqkb[:])
        wv_sb = consts.tile([P, KT, HPC * VW], BF16)
        nc.sync.dma_start(
            out=wv_sb, in_=wva.rearrange("p (kt m) -> p kt m", m=HPC * VW)
        )
        # x split across the two hwdge queues: kt0/1 behind the weights on
        # SP, kt2/3 on the ACT queue (idle during the ramp) in parallel.
        x_sb = consts.tile([P, KT, T], BF16)
        for kt in range(2):
            nc.sync.dma_start(out=x_sb[:, kt, :], in_=xT[kt * P : (kt + 1) * P, :])
        for kt in range(2, KT):
            nc.scalar.dma_start(out=x_sb[:, kt, :], in_=xT[kt * P : (kt + 1) * P, :])
        wo_sb = consts.tile([P, 2, D], BF16)
        nc.sync.dma_start(out=wo_sb, in_=wo.rearrange("p (hp d) -> p hp d", d=D))

        triu_f = consts.tile([P, P], F32)
        make_upper_triangular(nc, triu_f, val=1.0, diag=True)
        triu = consts.tile([P, P], BF16)
        nc.vector.tensor_copy(triu, triu_f)
        ones64 = consts.tile([P, HD], BF16)
        nc.vector.memset(ones64, 1.0)

        qT_sb = qkv.tile([P, 2, T], BF16)
        kT_sb = qkv.tile([P, 2, T], BF16)
        v_sb = qkv.tile([P, NKT, HPC * VW], BF16)
        attn_pair = [
            qkv.tile([P, T], BF16, tag=f"ap{hp}", name=f"ap{hp}") for hp in range(2)
        ]

        # ---------- job helpers ------------------------------------------
        def evac(ps_ap, out_ap, cols, bias=None):
            """PSUM->SBUF evacuation on whichever engine is less loaded."""
            ca, cd = asn.copy_cost(cols)
            if asn.pick(ca, cd) == "act":
                nc.scalar.activation(
                    out=out_ap, in_=ps_ap, func=IDENT,
                    bias=bias if bias is not None else 0.0,
                )
            else:
                if bias is not None:
                    nc.vector.tensor_scalar_add(out_ap, ps_ap, bias)
                else:
                    nc.vector.tensor_copy(out_ap, ps_ap)

        def qk_job(wi, w_sb, dst, m, nb):
            """One [128,1024] column block of the q or k projection."""
            ps = scps.tile([P, AB], F32, tag="sc", name="qk_ps")
            for lo in range(0, AB, 512):
                for kt in range(KT):
                    nc.tensor.matmul(
                        ps[:, lo : lo + 512],
                        lhsT=w_sb[:, kt, m * P : (m + 1) * P],
                        rhs=x_sb[:, kt, nb * AB + lo : nb * AB + lo + 512],
                        start=(kt == 0),
                        stop=(kt == KT - 1),
                    )
            evac(
                ps, dst[:, m, nb * AB : (nb + 1) * AB], AB,
                bias=wqkb_sb[:, 2 * wi + m : 2 * wi + m + 1],
            )

        def v_job(tt):
            """V projection for key tile tt (+ ones columns)."""
            ps = scps.tile([P, AB], F32, tag="sc", name="v_ps")
            for kt in range(KT):
                nc.tensor.matmul(
                    ps[:, 0 : HPC * VW],
                    lhsT=x_sb[:, kt, tt * P : (tt + 1) * P],
                    rhs=wv_sb[:, kt, :],
                    start=(kt == 0),
                    stop=(kt == KT - 1),
                )
            evac(ps[:, 0 : HPC * VW], v_sb[:, tt, :], HPC * VW)
            ones_cols = v_sb[:, tt, :].rearrange("p (h w) -> p h w", w=VW)[:, :, HD]
            nc.vector.memset(ones_cols, 1.0)

        def outproj_job(tt):
            """y[tt*128:(tt+1)*128, :] = sum_hp attn_pair[hp][:, tt].T @ wo."""
            ps = scps.tile([P, AB], F32, tag="sc", name="y_ps")
            for hp in range(2):
                nc.tensor.matmul(
                    ps[:, 0:D],
                    lhsT=attn_pair[hp][:, tt * P : (tt + 1) * P],
                    rhs=wo_sb[:, hp, :],
                    start=(hp == 0),
                    stop=(hp == 1),
                )
            yt = work.tile([P, D], F32, tag="yt", bufs=3, name="yt")
            evac(ps[:, 0:D], yt, D)
            # the last four y writes land after all exp work: overlap them
            # across both hwdge queues (ACT queue is empty by then)
            eng = nc.scalar if tt >= 12 and tt % 2 == 1 else nc.sync
            eng.dma_start(out=y[tt * P : (tt + 1) * P, :], in_=yt)

        # ---------- attention block --------------------------------------
        def attention_block(hp, qb, bg_jobs, boundary_jobs=()):
            """Causal attention for head pair hp over query block qb.

            AV accumulators are split into per-512-column halves so each
            half's normalization chain runs as soon as its last kt lands —
            half 0's norm hides under the block's remaining kt steps, and
            the next block's AV for a given half only waits on that half's
            (already finished) norm.
            """
            nkt = (qb + 1) * (AB // P)
            khs = [kT_sb[0:HD, hp, :], kT_sb[HD : 2 * HD, hp, :]]
            qhs = [qT_sb[0:HD, hp, :], qT_sb[HD : 2 * HD, hp, :]]
            opss = {}  # (head, half) -> psum tile
            for li in range(2):
                for i in (0, 1):
                    opss[(i, li)] = avps.tile(
                        [VW, 512], F32, tag=f"av{i}{li}", name=f"ops{i}{li}"
                    )
            half_last = [
                min(nkt, qb * (AB // P) + (lo + 512) // P) - 1
                for lo in range(0, AB, 512)
            ]

            def norm_half(li):
                lo = li * 512
                for i in (0, 1):
                    ops = opss[(i, li)]
                    # unnormalized attn + den row -> SBUF bf16
                    ua = work.tile(
                        [VW, 512], BF16, tag=f"ua{i}", bufs=4, name=f"ua{i}"
                    )
                    evac(ops, ua, 512)
                    # broadcast the raw denominator row with a K=1 outer
                    # product, then reciprocal the broadcast tile (DVE cost
                    # is free-dim bound, so this costs the same as a row).
                    bc = avps.tile([VW, 512], F32, tag=f"av{i}{li}", name=f"bc{i}{li}")
                    nc.tensor.matmul(
                        bc[0:HD, :],
                        lhsT=ones64[HD : HD + 1, :],
                        rhs=ua[HD : HD + 1, :],
                        start=True,
                        stop=True,
                    )
                    rec = work.tile([VW, 512], F32, tag="rec", bufs=4, name="rec")
                    nc.vector.reciprocal_approx_fast(
                        out=rec[0:HD, :], in_=bc[0:HD, :]
                    )
                    asn.t_dve += 0.66
                    if i == 0:
                        dst = attn_pair[hp][0:HD, qb * AB + lo : qb * AB + lo + 512]
                    else:
                        dst = work.tile(
                            [HD, 512], BF16, tag="atmp", bufs=4, name="atmp"
                        )
                    nc.vector.tensor_mul(dst, ua[0:HD, :], rec[0:HD, :])
                    asn.t_dve += 0.66
                    if i == 1:
                        nc.sync.dma_start(
                            out=attn_pair[hp][
                                HD : 2 * HD, qb * AB + lo : qb * AB + lo + 512
                            ],
                            in_=dst,
                        )

            def do_exp(out_ap, in_ap, cols, force_act=False):
                ca, cd = asn.exp_cost(cols)
                if force_act or asn.pick(ca, cd) == "act":
                    if force_act:
                        asn.t_act += ca
                    nc.scalar.activation(
                        out=out_ap, in_=in_ap, func=EXP, scale=1.0 / np.sqrt(HD)
                    )
                else:
                    nc.vector.tensor_scalar(
                        out=out_ap.bitcast(I16), in0=in_ap,
                        scalar1=float(SCH_A), scalar2=float(SCH_B),
                        op0=mybir.AluOpType.mult, op1=mybir.AluOpType.add,
                    )

            def triu_mask(eT_view):
                # zero q < k inside the diagonal 128-block(s): free dims may be
                # [2, 128] (merged heads) or [128]
                pat = (
                    [[0, 2], [1, P]] if len(eT_view.shape) == 3 else [[1, P]]
                )
                nc.gpsimd.affine_select(
                    out=eT_view, in_=eT_view,
                    compare_op=mybir.AluOpType.is_ge, fill=0.0,
                    base=0, pattern=pat, channel_multiplier=-1,
                )

            def emit_scores(kt):
                """Score matmuls + exp + causal mask for one kt; returns the
                eT tile(s)."""
                off = max(0, kt * P - qb * AB)
                is_diag = off > 0 or kt * P == qb * AB
                near_tail = kt in (
                    half_last[0] - 1, half_last[0],
                    half_last[1] - 1, half_last[1],
                )
                if off >= 512:
                    # single-half step: pack both heads into one score tile,
                    # one exp instruction, one mask op
                    cols = AB - off
                    sps = scps.tile([P, AB], F32, tag="sc", name="sps")
                    for i in (0, 1):
                        nc.tensor.matmul(
                            sps[:, 512 * i : 512 * i + cols],
                            lhsT=khs[i][:, kt * P : (kt + 1) * P],
                            rhs=qhs[i][:, qb * AB + off : (qb + 1) * AB],
                            start=True,
                            stop=True,
                        )
                    eT = work.tile([P, AB], BF16, tag="eT", bufs=4, name="eT")
                    sview = sps.rearrange("p (h c) -> p h c", c=512)[:, :, 0:cols]
                    eview = eT.rearrange("p (h c) -> p h c", c=512)[:, :, 0:cols]
                    do_exp(eview, sview, 2 * cols, force_act=near_tail)
                    if is_diag:
                        triu_mask(
                            eT.rearrange("p (h c) -> p h c", c=512)[:, :, 0:P]
                        )
                    return (eT,)
                halves = [
                    (max(off, lo), lo + 512, li)
                    for li, lo in enumerate(range(0, AB, 512))
                ]
                eTs = []
                for i in (0, 1):
                    sps = scps.tile([P, AB], F32, tag="sc", name="sps")
                    for o, hi, _li in halves:
                        nc.tensor.matmul(
                            sps[:, o:hi],
                            lhsT=khs[i][:, kt * P : (kt + 1) * P],
                            rhs=qhs[i][:, qb * AB + o : qb * AB + hi],
                            start=True,
                            stop=True,
                        )
                    eT = work.tile([P, AB], BF16, tag="eT", bufs=4, name="eT")
                    do_exp(
                        eT[:, off:AB], sps[:, off:AB], AB - off,
                        force_act=near_tail,
                    )
                    if is_diag:
                        triu_mask(eT[:, off : off + P])
                    eTs.append(eT)
                return tuple(eTs)

            def emit_avs(kt, eTs):
                off = max(0, kt * P - qb * AB)
                if off >= 512:
                    (eT,) = eTs
                    for i in (0, 1):
                        nc.tensor.matmul(
                            opss[(i, 1)][:, off - 512 : 512],
                            lhsT=v_sb[:, kt, (2 * hp + i) * VW : (2 * hp + i + 1) * VW],
                            rhs=eT[:, 512 * i : 512 * i + (AB - off)],
                            start=(kt == 0),
                            stop=(kt == half_last[1]),
                        )
                    return
                halves = [
                    (max(off, lo), lo + 512, li)
                    for li, lo in enumerate(range(0, AB, 512))
                ]
                for i in (0, 1):
                    for o, hi, li in halves:
                        nc.tensor.matmul(
                            opss[(i, li)][:, o - li * 512 : hi - li * 512],
                            lhsT=v_sb[:, kt, (2 * hp + i) * VW : (2 * hp + i + 1) * VW],
                            rhs=eTs[i][:, o:hi],
                            start=(kt == 0),
                            stop=(kt == half_last[li]),
                        )

            # software pipeline: scores run one kt ahead of the AVs so the
            # PE queue always has independent work in front of an AV that is
            # waiting on its exp.
            pending = emit_scores(0)
            for kt in range(nkt):
                if kt + 1 < nkt:
                    nxt = emit_scores(kt + 1)
                else:
                    nxt = None
                emit_avs(kt, pending)
                pending = nxt
                for li in range(2):
                    if kt == half_last[li]:
                        norm_half(li)
                if bg_jobs:
                    bg_jobs.pop(0)()
            for j in boundary_jobs:  # PE filler across the half-1 norm tail
                j()

        # ---------- emission order ---------------------------------------
        # ramp: k/q for pair 0, first v tiles
        qk_job(1, wk_sb, kT_sb, 0, 0)
        qk_job(1, wk_sb, kT_sb, 0, 1)
        qk_job(0, wq_sb, qT_sb, 0, 0)
        qk_job(0, wq_sb, qT_sb, 0, 1)
        for tt in range(4):
            v_job(tt)

        jobs_00 = [lambda tt=tt: v_job(tt) for tt in range(4, 12)]
        attention_block(
            0, 0, jobs_00,
            boundary_jobs=[lambda: v_job(12), lambda: v_job(13)],
        )
        for j in jobs_00:  # any leftovers
            j()

        jobs_01 = [lambda tt=tt: v_job(tt) for tt in range(14, 16)]
        jobs_01 += [
            lambda a=a: qk_job(*a)
            for a in [
                (1, wk_sb, kT_sb, 1, 0), (1, wk_sb, kT_sb, 1, 1),
            ]
        ]
        attention_block(
            0, 1, jobs_01,
            boundary_jobs=[
                lambda: qk_job(0, wq_sb, qT_sb, 1, 0),
                lambda: qk_job(0, wq_sb, qT_sb, 1, 1),
            ],
        )
        for j in jobs_01:
            j()

        attention_block(
            1, 0, [],
            boundary_jobs=[lambda: outproj_job(0), lambda: outproj_job(1)],
        )

        jobs_11 = [lambda tt=tt: outproj_job(tt) for tt in range(2, 8)]
        attention_block(
            1, 1, jobs_11,
            boundary_jobs=[lambda: outproj_job(8), lambda: outproj_job(9)],
        )
        for j in jobs_11:
            j()
        for tt in range(10, 16):
            outproj_job(tt)

    nc.compile()
    return nc


def make_in_maps(x, W_qkv, b_qkv, W_out):
    x = np.asarray(x, np.float32)
    W_qkv = np.asarray(W_qkv, np.float32)
    b_qkv = np.asarray(b_qkv, np.float32)
    W_out = np.asarray(W_out, np.float32)
    in_maps = []
    for c in range(2 * B):
        b, g = divmod(c, 2)
        ch = g * HPC * HD
        wqkb = np.concatenate(
            [
                b_qkv[ch : ch + 256].reshape(2, P).T,
                b_qkv[D + ch : D + ch + 256].reshape(2, P).T,
            ],
            axis=1,
        )  # [128, 4]: cols = q-m0, q-m1, k-m0, k-m1
        wva = np.zeros((D, HPC * VW), np.float32)
        wva3 = wva.reshape(D, HPC, VW)
        wva3[:, :, :HD] = W_qkv[:, 2 * D + ch : 2 * D + ch + 256].reshape(D, HPC, HD)

        def pkm(a):  # [(kt p), m] -> [p, (kt m)] (on-device layout, contiguous DMA)
            m = a.shape[1]
            return np.ascontiguousarray(
                a.reshape(KT, P, m).transpose(1, 0, 2).reshape(P, KT * m)
            )

        wo_p = W_out[ch : ch + 256, :].reshape(2, P, D).transpose(1, 0, 2)
        in_maps.append(
            {
                "xT": np.ascontiguousarray(x[b].T).astype(_NP_BF16),
                "wqa": pkm(W_qkv[:, ch : ch + 256]).astype(_NP_BF16),
                "wka": pkm(W_qkv[:, D + ch : D + ch + 256]).astype(_NP_BF16),
                "wva": pkm(wva).astype(_NP_BF16),
                "wqkb": np.ascontiguousarray(wqkb, np.float32),
                "wo": np.ascontiguousarray(wo_p.reshape(P, 2 * D)).astype(_NP_BF16),
            }
        )
    return in_maps


def assemble(results, b_out, vbias_y):
    b_out = np.asarray(b_out, np.float32) + vbias_y
    out = np.empty((B, T, D), np.float32)
    for b in range(B):
        out[b] = results[2 * b]["y"] + results[2 * b + 1]["y"] + b_out[None, :]
    return out


_CACHE = {}


def kernel(x, W_qkv, b_qkv, W_out, b_out):
    if "nc" not in _CACHE:
        _CACHE["nc"] = build_bass()
    in_maps = make_in_maps(x, W_qkv, b_qkv, W_out)
    vbias_y = np.asarray(b_qkv, np.float32)[2 * D :] @ np.asarray(W_out, np.float32)
    res = run_bass_kernel_spmd(_CACHE["nc"], in_maps, list(range(2 * B)))
    return assemble(res.results, b_out, vbias_y)
